# revision 1
# baseline (speedup 1.0000x reference)
"""DGCN diffusion-graph-conv kernel for 8 Trainium2 NeuronCores.

Math (per the reference):
    support S = D^-1/2 (adj+I)^T D^-1/2  with D = diag(rowsum(adj+I))
    x_m = T_m(S) x0  (Chebyshev recurrence, K=3 -> m=0..3)
    out = sum_m x_m @ W_m + bias

Implementation strategy (data-parallel over batch, 4 batches/core):
    Rewrite out = sum_m T_m(S) (x0 @ W_m) and fold the Chebyshev
    coefficients into the weights:
        V0 = W0 - W2, V1 = W1 - 3*W3, V2 = 2*W2, V3 = 4*W3
        U_m = x0 @ V_m   (projection; contracts feature dim d)
        out = U0 + S*(U1 + S*(U2 + S*U3))   (Horner; contracts node dim n)
    The projection's stationary operand is x0^T, which the host supplies
    directly (layout prep during sharding).  All matmuls run in fp32r
    (fp22 multiply / fp32 accumulate) at full PE rate.
"""

import numpy as np

import concourse.bacc as bacc
import concourse.tile as tile
import concourse.mybir as mybir
from concourse.bass_utils import run_bass_kernel_spmd

F32 = mybir.dt.float32
F32R = mybir.dt.float32r
AX = mybir.AxisListType
ALU = mybir.AluOpType

N_CORES = 8
B, N, D = 32, 512, 768
BL = B // N_CORES          # local batches per core = 4
BN = BL * N                # local rows = 2048
NT = BN // 128             # 16 row tiles
DT = D // 128              # 6 feature tiles
JT = N // 128              # 4 node tiles
WE = 256                   # output-column block width
EB = D // WE               # 3 column blocks


def _build_program():
    nc = bacc.Bacc("TRN2", target_bir_lowering=False, debug=False,
                   num_devices=N_CORES)
    # x0^T for this core: [d, (b n)]
    inpT_d = nc.dram_tensor("inpT", [D, BN], F32, kind="ExternalInput").ap()
    adj_d = nc.dram_tensor("adj", [N, N], F32, kind="ExternalInput").ap()
    wts_d = nc.dram_tensor("wts", [D * 4, D], F32, kind="ExternalInput").ap()
    bias_d = nc.dram_tensor("bias", [D], F32, kind="ExternalInput").ap()
    eye_d = nc.dram_tensor("eye", [128, 128], F32, kind="ExternalInput").ap()
    out_d = nc.dram_tensor("out", [BN, D], F32, kind="ExternalOutput").ap()
    dscr = nc.dram_tensor("dscr", [N], F32)

    # weights viewed as [m, d, e] (reference row index is d*4+m)
    wts_v = wts_d.rearrange("(d m) e -> m d e", m=4)

    with tile.TileContext(nc) as tc:
        with (
            tc.tile_pool(name="const", bufs=1) as constp,
            tc.tile_pool(name="sup", bufs=1) as supp,
            tc.tile_pool(name="x0T", bufs=1) as x0Tp,
            tc.tile_pool(name="wst", bufs=12) as wp,
            tc.tile_pool(name="vt", bufs=24) as vp,
            tc.tile_pool(name="ut", bufs=25) as up,
            tc.tile_pool(name="pg", bufs=7) as pgp,
            tc.tile_pool(name="stg", bufs=4) as stgp,
            tc.tile_pool(name="ps", bufs=8, space="PSUM") as psp,
        ):
            def load_v(eb, dts=None, v=None):
                """DMA the W column block and build the V combos."""
                c0 = eb * WE
                if v is None:
                    v = [[None] * DT for _ in range(2)]
                for dt in (dts if dts is not None else range(DT)):
                    w_raw = [None] * 4
                    for m in (0, 2, 1, 3):
                        w = wp.tile([128, WE], F32,
                                    name=f"w{eb}_{dt}_{m}", tag="wt")
                        nc.sync.dma_start(
                            w[:],
                            wts_v[m, dt * 128:(dt + 1) * 128, c0:c0 + WE])
                        w_raw[m] = w[:]
                    vp01 = vp.tile([128, 2, WE], F32R,
                                   name=f"v{eb}_{dt}_01", tag="vt")
                    nc.vector.tensor_sub(vp01[:, 0, :], w_raw[0], w_raw[2])
                    nc.vector.scalar_tensor_tensor(
                        vp01[:, 1, :], w_raw[3], -3.0, w_raw[1],
                        ALU.mult, ALU.add)
                    vp23 = vp.tile([128, 2, WE], F32R,
                                   name=f"v{eb}_{dt}_23", tag="vt")
                    nc.vector.tensor_scalar_mul(vp23[:, 0, :], w_raw[2], 2.0)
                    nc.vector.tensor_scalar_mul(vp23[:, 1, :], w_raw[3], 4.0)
                    v[0][dt], v[1][dt] = vp01, vp23
                return v

            eye128 = constp.tile([128, 128], F32)
            nc.gpsimd.dma_start(eye128[:], eye_d[:])

            # ---- DMA issue order: first-needed first ----
            # x0^T chunk 0 (row tiles bt=0..3), then eb0 weights, then the
            # rest of x0^T, then support/bias inputs.
            x0T = []
            for dt in range(DT):
                t = x0Tp.tile([128, BN], F32R, name=f"x0T{dt}")
                x0T.append(t)
            adjts = []
            for t in range(JT):
                adjt = supp.tile([128, N], F32, name=f"adjt{t}")
                nc.gpsimd.dma_start(adjt[:], adj_d[t * 128:(t + 1) * 128, :])
                adjts.append(adjt)

            # interleave eb0 weights with the first x0^T chunks in the order
            # the first projection consumes them
            v_cur = None
            for dt in range(DT):
                nc.sync.dma_start(
                    x0T[dt][:, 0:256],
                    inpT_d[dt * 128:(dt + 1) * 128, 0:256].bitcast(F32R))
                v_cur = load_v(0, dts=[dt], v=v_cur)

            for dt in range(DT):
                nc.sync.dma_start(
                    x0T[dt][:, 256:512],
                    inpT_d[dt * 128:(dt + 1) * 128, 256:512].bitcast(F32R))
            for ck in range(1, 4):
                for dt in range(DT):
                    eng = nc.gpsimd if ck == 3 else nc.sync
                    eng.dma_start(
                        x0T[dt][:, ck * 512:(ck + 1) * 512],
                        inpT_d[dt * 128:(dt + 1) * 128,
                               ck * 512:(ck + 1) * 512].bitcast(F32R))

            bias_bc = constp.tile([128, D], F32)
            nc.gpsimd.dma_start(
                bias_bc[:], bias_d.unsqueeze(0).broadcast_to([128, D]))

            # ---- support matrix S^T = (adj+I) * d[j]d[i], built as
            #      adj*d[j]d[i] plus a diagonal d^2 fix-up ----
            dcols, dsqs = [], []
            for t in range(JT):
                adjt = adjts[t]
                rs = supp.tile([128, 1], F32, name=f"rs{t}", tag="rs",
                               bufs=2)
                nc.vector.tensor_reduce(rs[:], adjt[:], axis=AX.X, op=ALU.add)
                nc.vector.tensor_scalar_add(rs[:], rs[:], 1.0)
                sq = supp.tile([128, 1], F32, name=f"sq{t}", tag="sq",
                               bufs=2)
                nc.scalar.sqrt(sq[:], rs[:])
                dcol = supp.tile([128, 1], F32, name=f"dcol{t}")
                nc.vector.reciprocal(dcol[:], sq[:])
                dsq = supp.tile([128, 1], F32, name=f"dsq{t}")
                nc.vector.tensor_mul(dsq[:], dcol[:], dcol[:])
                nc.gpsimd.dma_start(dscr.ap()[t * 128:(t + 1) * 128],
                                    dcol[:])
                dcols.append(dcol)
                dsqs.append(dsq)
            dbc = constp.tile([128, N], F32)
            nc.gpsimd.dma_start(
                dbc[:], dscr.ap().unsqueeze(0).broadcast_to([128, N]))
            st_t = []
            for t in range(JT):
                s = supp.tile([128, N], F32R, name=f"st{t}")
                nc.vector.scalar_tensor_tensor(
                    s[:], adjts[t][:], dcols[t][:], dbc[:],
                    ALU.mult, ALU.mult)
                diagfix = supp.tile([128, 128], F32, name=f"dfix{t}",
                                    tag="dfix", bufs=2)
                nc.vector.tensor_scalar_mul(diagfix[:], eye128[:], dsqs[t][:])
                nc.vector.tensor_add(
                    s[:, t * 128:(t + 1) * 128],
                    s[:, t * 128:(t + 1) * 128], diagfix[:])
                st_t.append(s)

            # ---- main loops: per column-block project then Horner ----
            for eb in range(EB):
                c0 = eb * WE
                v = v_cur

                def proj(b, u=None):
                    # projection for batch b; U stored in batch-pair tiles
                    # [128, 2, WE] (dim1 = b parity) shared with b^1
                    h = b % 2
                    if u is None:
                        u = [[None] * JT for _ in range(4)]
                        for m in range(4):
                            for nt in range(JT):
                                u[m][nt] = up.tile(
                                    [128, 2, WE], F32R,
                                    name=f"u{eb}_{b // 2}_{nt}_{m}",
                                    tag="ut")
                    for nt in range(JT):
                        bt = b * JT + nt
                        for pr in range(2):
                            pmt = psp.tile([128, 2, WE], F32,
                                           name=f"pp{eb}_{bt}_{pr}",
                                           tag="ps")
                            for dt in range(DT):
                                lhs = x0T[dt][:, bt * 128:(bt + 1) * 128]
                                nc.tensor.matmul(
                                    pmt[:], lhs, v[pr][dt][:],
                                    start=(dt == 0), stop=(dt == DT - 1))
                            for half in range(2):
                                m = pr * 2 + half
                                if m == 0:
                                    nc.vector.tensor_add(
                                        u[m][nt][:, h, :], pmt[:, 0, :],
                                        bias_bc[:, c0:c0 + WE])
                                else:
                                    nc.scalar.copy(
                                        u[m][nt][:, h, :], pmt[:, half, :])
                    return u

                def horner(bp, u):
                    # Horner for batch pair bp (b = 2*bp, 2*bp+1), N=512
                    # matmuls over the pair dim.  P2 -> fresh tiles (u[3] is
                    # still read by later-traced matmuls), P1 -> u[3],
                    # out -> staged + one strided DMA per nt
                    src_t = u[3]
                    for step, (madd, dest) in enumerate(
                            [(2, "fresh"), (1, 3), (0, None)]):
                        new_t = [None] * JT
                        for nt in range(JT):
                            ph = psp.tile([128, 2, WE], F32,
                                          name=f"phh{eb}_{bp}_{step}_{nt}",
                                          tag="ps")
                            for jt in range(JT):
                                nc.tensor.matmul(
                                    ph[:],
                                    st_t[jt][:, nt * 128:(nt + 1) * 128],
                                    src_t[jt][:],
                                    start=(jt == 0), stop=(jt == JT - 1))
                            if dest == "fresh":
                                pgt = pgp.tile([128, 2, WE], F32R,
                                               name=f"pg{eb}_{bp}_{nt}",
                                               tag="pg")
                                nc.vector.tensor_add(
                                    pgt[:], ph[:], u[madd][nt][:])
                                new_t[nt] = pgt
                            elif dest is not None:
                                nc.vector.tensor_add(
                                    u[dest][nt][:], ph[:], u[madd][nt][:])
                                new_t[nt] = u[dest][nt]
                            else:
                                so = stgp.tile([128, 2, WE], F32,
                                               name=f"so{eb}_{bp}_{nt}",
                                               tag="outst")
                                nc.vector.tensor_add(
                                    so[:], ph[:], u[0][nt][:])
                                r0 = (2 * bp * JT + nt) * 128
                                nc.sync.dma_start(
                                    out_d.rearrange(
                                        "(x p) e -> p x e", p=128)[
                                        :, r0 // 128:r0 // 128 + 5:4,
                                        c0:c0 + WE],
                                    so[:])
                        src_t = new_t

                # software pipeline: keep independent projection work
                # available while each Horner chain waits on evictions
                u0p = proj(0)
                u0p = proj(1, u0p)
                if eb + 1 < EB:
                    v_next = load_v(eb + 1)
                u1p = proj(2)
                horner(0, u0p)
                u1p = proj(3, u1p)
                horner(1, u1p)
                if eb + 1 < EB:
                    v_cur = v_next
    nc.compile()
    return nc


_CACHE = {}


def _get_program():
    if "nc" not in _CACHE:
        _CACHE["nc"] = _build_program()
    return _CACHE["nc"]


def make_in_maps(inputs, adj, weights, biases):
    inputs = np.ascontiguousarray(inputs, dtype=np.float32)
    adj = np.ascontiguousarray(adj, dtype=np.float32)
    weights = np.ascontiguousarray(weights, dtype=np.float32)
    biases = np.ascontiguousarray(biases, dtype=np.float32)
    assert inputs.shape == (B, N, D)
    assert adj.shape == (N, N)
    assert weights.shape == (D * 4, D)
    assert biases.shape == (D,)
    eye = np.eye(128, dtype=np.float32)
    in_maps = []
    for c in range(N_CORES):
        x0T = np.ascontiguousarray(
            inputs[c * BL:(c + 1) * BL].reshape(BN, D).T)
        in_maps.append({
            "inpT": x0T,
            "adj": adj,
            "wts": weights,
            "bias": biases,
            "eye": eye,
        })
    return in_maps


def kernel(inputs, adj, weights, biases):
    nc = _get_program()
    in_maps = make_in_maps(inputs, adj, weights, biases)
    res = run_bass_kernel_spmd(nc, in_maps, list(range(N_CORES)))
    out = np.concatenate(
        [res.results[c]["out"].reshape(BL, N, D) for c in range(N_CORES)],
        axis=0)
    return out



# revision 2
# speedup vs baseline: 1.3115x; 1.3115x over previous
"""DGCN diffusion-graph-conv kernel for 8 Trainium2 NeuronCores (v2).

Math (per the reference):
    support S = D^-1/2 (adj+I)^T D^-1/2,  D = diag(rowsum(adj+I))
    x_m = T_m(S) x0  (Chebyshev, K=3),  out = sum_m x_m @ W_m + bias

Folding the Chebyshev coefficients into the weights
    V0 = W0 - W2, V1 = W1 - 3*W3, V2 = 2*W2, V3 = 4*W3
gives out_b = sum_{m=0..3} S^m (X_b @ V_m).

Per-core plan (data-parallel over batch, 4 batches/core):
    1.  Build S^T from adj on-chip; PE-transpose it to get S tiles;
        compute (S^T)^2 and (S^T)^3 in fp32r.
    2.  U0 = X @ V0 + bias in bf16xbf16 matmuls (error-critical path).
    3.  U_m = X @ V_m (m=1..3) in fp8e4m3 with DoubleRow perf mode
        (two 128-deep K-tiles per pass, ~1.7x fp32r throughput).
        Errors here are suppressed ~20x by the later S^m contraction.
    4.  out = U0 + [S|S^2|S^3]-apply over stacked U (fp8 DoubleRow).
All fp8 tensors carry power-of-2 scales (X:16, V:32, U:8, S^m:256);
the combined descale 2^-11 is folded into the final eviction.
"""

import numpy as np
import ml_dtypes

import concourse.bacc as bacc
import concourse.tile as tile
import concourse.mybir as mybir
from concourse.bass_utils import run_bass_kernel_spmd

F32 = mybir.dt.float32
F32R = mybir.dt.float32r
BF16 = mybir.dt.bfloat16
F8 = mybir.dt.float8e4
AX = mybir.AxisListType
ALU = mybir.AluOpType
DR = mybir.MatmulPerfMode.DoubleRow

N_CORES = 8
B, N, D = 32, 512, 768
BL = B // N_CORES          # batches per core = 4
BN = BL * N                # rows per core = 2048
NT = BN // 128             # 16 row blocks
DT = D // 128              # 6 feature tiles
DP = DT // 2               # 3 feature-tile pairs
JT = N // 128              # 4 node tiles
EC = 384                   # output-column chunk (psum-bank safe)

SX, SV, SU, SP = 16.0, 32.0, 8.0, 256.0
DESCALE = 1.0 / (SP * SU)          # 2^-11
U8SCALE = SU / (SX * SV)           # 1/64


def _build_program():
    nc = bacc.Bacc("TRN2", target_bir_lowering=False, debug=False,
                   num_devices=N_CORES)
    inpT_d = nc.dram_tensor("inpT", [D, BN], BF16, kind="ExternalInput").ap()
    adj_d = nc.dram_tensor("adj", [N, N], F32, kind="ExternalInput").ap()
    wts_d = nc.dram_tensor("wts", [D * 4, D], F32, kind="ExternalInput").ap()
    bias_d = nc.dram_tensor("bias", [D], F32, kind="ExternalInput").ap()
    eye_d = nc.dram_tensor("eye", [128, 128], F32, kind="ExternalInput").ap()
    out_d = nc.dram_tensor("out", [BN, D], F32, kind="ExternalOutput").ap()
    dscr = nc.dram_tensor("dscr", [N], F32)

    wts_v = wts_d.rearrange("(d m) e -> m d e", m=4)

    with tile.TileContext(nc) as tc:
        with (
            tc.tile_pool(name="const", bufs=1) as constp,
            tc.tile_pool(name="x0", bufs=1) as x0p,
            tc.tile_pool(name="x8", bufs=1) as x8p,
            tc.tile_pool(name="wraw", bufs=8) as wp,
            tc.tile_pool(name="v0", bufs=1) as v0p,
            tc.tile_pool(name="v8", bufs=1) as v8p,
            tc.tile_pool(name="vtmp", bufs=2) as vtp,
            tc.tile_pool(name="sup", bufs=1) as supp,
            tc.tile_pool(name="pt8", bufs=1) as pt8p,
            tc.tile_pool(name="u0", bufs=1) as u0p,
            tc.tile_pool(name="u8", bufs=1) as u8p,
            tc.tile_pool(name="outst", bufs=4) as outp,
            tc.tile_pool(name="psA", bufs=6, space="PSUM") as psA,
            tc.tile_pool(name="psT", bufs=2, space="PSUM") as psT,
        ):
            # ---------------- DMA issue ----------------
            eye = constp.tile([128, 128], F32R)
            nc.gpsimd.dma_start(eye[:], eye_d[:].bitcast(F32R))
            adjts = []
            for t in range(JT):
                a = supp.tile([128, N], F32, name=f"adj{t}")
                nc.gpsimd.dma_start(a[:], adj_d[t * 128:(t + 1) * 128, :])
                adjts.append(a)

            # W0/W2 first (gate V0 -> U0), then W1/W3
            wtiles = {}
            for m in (0, 2):
                for dt in range(DT):
                    w = wp.tile([128, D], F32, name=f"w{m}_{dt}", tag="wt")
                    nc.sync.dma_start(
                        w[:], wts_v[m, dt * 128:(dt + 1) * 128, :])
                    wtiles[(m, dt)] = w

            x0 = []
            for dt in range(DT):
                x = x0p.tile([128, BN], BF16, name=f"x0_{dt}")
                x0.append(x)
            for ck in range(4):
                for dt in range(DT):
                    nc.gpsimd.dma_start(
                        x0[dt][:, ck * 512:(ck + 1) * 512],
                        inpT_d[dt * 128:(dt + 1) * 128,
                               ck * 512:(ck + 1) * 512])

            for m in (1, 3):
                for dt in range(DT):
                    w = wp.tile([128, D], F32, name=f"w{m}_{dt}", tag="wt")
                    nc.sync.dma_start(
                        w[:], wts_v[m, dt * 128:(dt + 1) * 128, :])
                    wtiles[(m, dt)] = w

            bias_bc = constp.tile([128, D], F32)
            nc.gpsimd.dma_start(
                bias_bc[:], bias_d.unsqueeze(0).broadcast_to([128, D]))

            # ---------------- support S^T ----------------
            dcols, dsqs = [], []
            for t in range(JT):
                rs = supp.tile([128, 1], F32, name=f"rs{t}", tag="rs", bufs=2)
                nc.vector.tensor_reduce(rs[:], adjts[t][:], axis=AX.X,
                                        op=ALU.add)
                nc.vector.tensor_scalar_add(rs[:], rs[:], 1.0)
                sq = supp.tile([128, 1], F32, name=f"sq{t}", tag="sq", bufs=2)
                nc.scalar.sqrt(sq[:], rs[:])
                dcol = supp.tile([128, 1], F32, name=f"dcol{t}")
                nc.vector.reciprocal(dcol[:], sq[:])
                dsq = supp.tile([128, 1], F32, name=f"dsq{t}")
                nc.vector.tensor_mul(dsq[:], dcol[:], dcol[:])
                nc.gpsimd.dma_start(dscr.ap()[t * 128:(t + 1) * 128], dcol[:])
                dcols.append(dcol)
                dsqs.append(dsq)
            dbc = constp.tile([128, N], F32)
            nc.gpsimd.dma_start(
                dbc[:], dscr.ap().unsqueeze(0).broadcast_to([128, N]))
            st_t = []
            for t in range(JT):
                s = supp.tile([128, N], F32R, name=f"st{t}")
                nc.vector.scalar_tensor_tensor(
                    s[:], adjts[t][:], dcols[t][:], dbc[:], ALU.mult, ALU.mult)
                dfix = supp.tile([128, 128], F32, name=f"dfix{t}",
                                 tag="dfix", bufs=2)
                nc.vector.tensor_scalar_mul(dfix[:], eye[:].bitcast(F32),
                                            dsqs[t][:])
                nc.vector.tensor_add(
                    s[:, t * 128:(t + 1) * 128],
                    s[:, t * 128:(t + 1) * 128], dfix[:])
                st_t.append(s)

            # ---------------- X8 quantize (x0 * 16 -> fp8) ----------------
            x8 = []
            for dp in range(DP):
                t8 = x8p.tile([128, 2, BN], F8, name=f"x8_{dp}")
                nc.scalar.mul(t8[:, 0, :], x0[2 * dp][:], SX)
                nc.vector.tensor_scalar_mul(t8[:, 1, :], x0[2 * dp + 1][:], SX)
                x8.append(t8)

            # ---------------- V0 (bf16) ----------------
            v0 = []
            for dt in range(DT):
                v = v0p.tile([128, D], BF16, name=f"v0_{dt}")
                nc.vector.tensor_sub(v[:], wtiles[(0, dt)][:],
                                     wtiles[(2, dt)][:])
                v0.append(v)

            # ---------------- PE: transpose S^T -> S ----------------
            s_t = [supp.tile([128, N], F32R, name=f"s{t}") for t in range(JT)]
            for src in range(JT):
                for dst in range(JT):
                    pt = psT.tile([128, 128], F32R, name=f"pt{src}_{dst}",
                                  tag="pt")
                    nc.tensor.transpose(
                        pt[:], st_t[src][:, dst * 128:(dst + 1) * 128], eye[:])
                    nc.scalar.copy(s_t[dst][:, src * 128:(src + 1) * 128],
                                   pt[:])

            # ---------------- PE: powers (S^T)^2, (S^T)^3 ----------------
            pt8 = {}
            for m in (1, 2, 3):
                for u in range(2):
                    pt8[(m, u)] = pt8p.tile([128, 2, N], F8,
                                            name=f"pt8_{m}_{u}")
            for u in range(2):
                for i in range(2):
                    nc.vector.tensor_scalar_mul(
                        pt8[(1, u)][:, i, :], st_t[2 * u + i][:], SP)
            st2 = [supp.tile([128, N], F32R, name=f"st2_{t}")
                   for t in range(JT)]
            for ab in range(JT):
                ps = psA.tile([128, 512], F32, name=f"p2_{ab}", tag="ps")
                for cb in range(JT):
                    nc.tensor.matmul(
                        ps[:], s_t[cb][:, ab * 128:(ab + 1) * 128],
                        st_t[cb][:], start=(cb == 0), stop=(cb == JT - 1))
                nc.scalar.copy(st2[ab][:], ps[:])
                nc.vector.tensor_scalar_mul(
                    pt8[(2, ab // 2)][:, ab % 2, :], ps[:], SP)
            for ab in range(JT):
                ps = psA.tile([128, 512], F32, name=f"p3_{ab}", tag="ps")
                for cb in range(JT):
                    nc.tensor.matmul(
                        ps[:], s_t[cb][:, ab * 128:(ab + 1) * 128],
                        st2[cb][:], start=(cb == 0), stop=(cb == JT - 1))
                nc.vector.tensor_scalar_mul(
                    pt8[(3, ab // 2)][:, ab % 2, :], ps[:], SP)

            # ---------------- V8 (m=1..3, fp8 * 32) ----------------
            v8 = {}
            for m in (1, 2, 3):
                for dp in range(DP):
                    v8[(m, dp)] = v8p.tile([128, 2, D], F8,
                                           name=f"v8_{m}_{dp}")
            for dp in range(DP):
                for i in range(2):
                    dt = 2 * dp + i
                    tmp = vtp.tile([128, D], F32, name=f"vt_{dt}", tag="vt")
                    nc.vector.scalar_tensor_tensor(
                        tmp[:], wtiles[(3, dt)][:], -3.0, wtiles[(1, dt)][:],
                        ALU.mult, ALU.add)
                    nc.scalar.mul(v8[(1, dp)][:, i, :], tmp[:], SV)
                    nc.scalar.mul(v8[(2, dp)][:, i, :], wtiles[(2, dt)][:],
                                  2.0 * SV)
                    nc.scalar.mul(v8[(3, dp)][:, i, :], wtiles[(3, dt)][:],
                                  4.0 * SV)

            # ---------------- U0 = X @ V0 + bias (bf16) ----------------
            u0 = []
            for rb in range(NT):
                ut = u0p.tile([128, D], BF16, name=f"u0_{rb}")
                u0.append(ut)
                pss = [psA.tile([128, 512], F32, name=f"pu0_{rb}_{e}",
                                tag="ps") for e in range(2)]
                for dt in range(DT):
                    lhs = x0[dt][:, rb * 128:(rb + 1) * 128]
                    for e in range(2):
                        nc.tensor.matmul(
                            pss[e][:, 0:EC], lhs,
                            v0[dt][:, e * EC:(e + 1) * EC],
                            start=(dt == 0), stop=(dt == DT - 1))
                for e in range(2):
                    nc.vector.tensor_add(
                        ut[:, e * EC:(e + 1) * EC], pss[e][:, 0:EC],
                        bias_bc[:, e * EC:(e + 1) * EC])

            # ---------------- U_m = X @ V_m (fp8 DoubleRow) ----------------
            u8 = {}
            for m in (1, 2, 3):
                for b in range(BL):
                    for u in range(2):
                        u8[(m, b, u)] = u8p.tile(
                            [128, 2, D], F8, name=f"u8_{m}_{b}_{u}")
            for rb in range(NT):
                b, jt = rb // JT, rb % JT
                u, i = jt // 2, jt % 2
                for m in (1, 2, 3):
                    pss = [psA.tile([128, 512], F32, name=f"pu{m}_{rb}_{e}",
                                    tag="ps") for e in range(2)]
                    for dp in range(DP):
                        lhs = x8[dp][:, :, rb * 128:(rb + 1) * 128]
                        for e in range(2):
                            nc.tensor.matmul(
                                pss[e][:, 0:EC], lhs,
                                v8[(m, dp)][:, :, e * EC:(e + 1) * EC],
                                start=(dp == 0), stop=(dp == DP - 1),
                                perf_mode=DR)
                    dst = u8[(m, b, u)]
                    for e in range(2):
                        if (rb + m) % 2 == 0:
                            nc.scalar.mul(dst[:, i, e * EC:(e + 1) * EC],
                                          pss[e][:, 0:EC], U8SCALE)
                        else:
                            nc.vector.tensor_scalar_mul(
                                dst[:, i, e * EC:(e + 1) * EC],
                                pss[e][:, 0:EC], U8SCALE)

            # ---------------- apply + final eviction ----------------
            MP = [(1, 0), (1, 1), (2, 0), (2, 1), (3, 0), (3, 1)]
            for b in range(BL):
                for nb in range(JT):
                    rb = b * JT + nb
                    pss = [psA.tile([128, 512], F32, name=f"pa_{rb}_{e}",
                                    tag="ps") for e in range(2)]
                    for k, (m, u) in enumerate(MP):
                        lhs = pt8[(m, u)][:, :, nb * 128:(nb + 1) * 128]
                        for e in range(2):
                            nc.tensor.matmul(
                                pss[e][:, 0:EC], lhs,
                                u8[(m, b, u)][:, :, e * EC:(e + 1) * EC],
                                start=(k == 0), stop=(k == len(MP) - 1),
                                perf_mode=DR)
                    so = outp.tile([128, D], F32, name=f"so_{rb}", tag="so")
                    for e in range(2):
                        nc.vector.scalar_tensor_tensor(
                            so[:, e * EC:(e + 1) * EC], pss[e][:, 0:EC],
                            DESCALE, u0[rb][:, e * EC:(e + 1) * EC],
                            ALU.mult, ALU.add)
                    nc.sync.dma_start(
                        out_d[rb * 128:(rb + 1) * 128, :], so[:])
    nc.compile()
    return nc


_CACHE = {}


def _get_program():
    if "nc" not in _CACHE:
        _CACHE["nc"] = _build_program()
    return _CACHE["nc"]


def make_in_maps(inputs, adj, weights, biases):
    inputs = np.ascontiguousarray(inputs, dtype=np.float32)
    adj = np.ascontiguousarray(adj, dtype=np.float32)
    weights = np.ascontiguousarray(weights, dtype=np.float32)
    biases = np.ascontiguousarray(biases, dtype=np.float32)
    assert inputs.shape == (B, N, D)
    assert adj.shape == (N, N)
    assert weights.shape == (D * 4, D)
    assert biases.shape == (D,)
    eye = np.eye(128, dtype=np.float32)
    in_maps = []
    for c in range(N_CORES):
        x0T = np.ascontiguousarray(
            inputs[c * BL:(c + 1) * BL].reshape(BN, D).T).astype(
                ml_dtypes.bfloat16)
        in_maps.append({
            "inpT": x0T,
            "adj": adj,
            "wts": weights,
            "bias": biases,
            "eye": eye,
        })
    return in_maps


def kernel(inputs, adj, weights, biases):
    nc = _get_program()
    in_maps = make_in_maps(inputs, adj, weights, biases)
    res = run_bass_kernel_spmd(nc, in_maps, list(range(N_CORES)))
    out = np.concatenate(
        [res.results[c]["out"].reshape(BL, N, D) for c in range(N_CORES)],
        axis=0)
    return out
